# revision 12
# baseline (speedup 1.0000x reference)
"""Trainium2 Bass kernel for LocalMQA (windowed multi-head attention block).

Data-parallel over (batch, sequence): each of 8 cores owns 1024 consecutive
query tokens (2 buckets of W=512) of one batch element.  The 512-token K/V
halo is NOT shipped from the host: each core uploads only its own tokens and
the halo is exchanged on-device with a ReduceScatter trick (core c places its
last 512 token columns into slot (c+1)%8 of an 8-slot buffer, all other slots
zeroed via shipped one-hot flags; RS(add) then delivers exactly the
predecessor's tail to every core at a fixed offset).  Per-call wire traffic
is the dominant cost under the axon tunnel, so input is the d-major bf16 own
shard plus 9 tiny flag columns, and output is bf16.

Weights, scales and band masks are baked into the NEFF as Const tensors.

Per-core pipeline (bf16 matmuls, fp32 PSUM):
  1. k/v/q projections; qk l2norm via ACT Square + PE ones-matmul + ACT
     Rsqrt + PE outer-product broadcast.  Halo k/v recomputed locally from
     the exchanged x tail.  Own-region work is emitted first so the PE stays
     busy while the collective completes.
  2. Windowed attention computed transposed (simT[j,i] = k_j.q_i) and BANDED:
     for window chunk jc only the i-columns that can be valid are computed
     (62.5% of the full rectangle); only the diagonal 128x128 block needs a
     mask multiply (lower/upper triangular consts), except bucket 0's halo
     chunks which use a full-width mask pre-multiplied by the halo-valid
     flag.  Softmax without max-subtraction (|sim| <= 8); denominator via PE
     ones-matmul.
  3. Per-head sigmoid gates; all gate rows staged to partition 0 with one
     SBUF->SBUF DMA so the epilogue needs no per-head DMAs.
  4. Output projection accumulating over heads, staged into [128, 2048]
     tiles, one DMA per 128-token block.
"""

import hashlib
import sys

import numpy as np
import ml_dtypes

try:
    import concourse.bass as bass  # noqa: F401
except ImportError:  # pragma: no cover
    sys.path.insert(0, "/opt/trn_rl_repo")

import concourse.bass as bass
import concourse.tile as tile
from concourse import bacc, mybir

BF = ml_dtypes.bfloat16
B, N, D = 2, 4096, 2048
H, DH, W = 8, 128, 512
SCALE = 8.0
NCORES = 8
TOK = (B * N) // NCORES          # 1024 own tokens per core
EXT = TOK + W                    # 1536 tokens incl. halo
DC = D // 128                    # 16 d-chunks
NBL = TOK // W                   # 2 buckets per core
NFLAG = 9                        # 8 one-hot RS slot flags + halo-valid
BFD = mybir.dt.bfloat16
F32 = mybir.dt.float32

# banded-attention geometry: per window chunk jc, the computed query-column
# range [IOFF, IOFF+WID) and the start of the in-slice diagonal block
IOFF = [0, 0, 0, 0, 0, 128, 256, 384]
WID = [128, 256, 384, 512, 512, 384, 256, 128]
DIAG = [0, 128, 256, 384, 0, 0, 0, 0]
B0OFF = [0, 128, 384, 768]       # offsets of the bl=0 full masks in cB/mask0


def _r128(ap):
    """(K, F) dram AP -> (128, K//128, F) partition-major view."""
    return ap.rearrange("(po pi) f -> pi po f", pi=128)


def _const_arrays(Wq, Wkv, q_scale, k_scale, Wg, bg, Wo):
    wqt = np.ascontiguousarray(np.asarray(Wq, np.float32).T).astype(BF)
    wkt = np.ascontiguousarray(
        np.asarray(Wkv[:H * DH], np.float32).T).astype(BF)
    wvt = np.ascontiguousarray(
        np.asarray(Wkv[H * DH:], np.float32).T).astype(BF)
    wgt = np.ascontiguousarray(np.asarray(Wg, np.float32).T).astype(BF)
    wot = np.ascontiguousarray(np.asarray(Wo, np.float32).T).astype(BF)
    qs = (np.asarray(q_scale, np.float32) * SCALE).reshape(1, DH).astype(BF)
    ks = np.asarray(k_scale, np.float32).reshape(1, DH).astype(BF)
    bgc = np.ascontiguousarray(np.asarray(bg, np.float32).reshape(H, 1))

    ii = np.arange(128)[None, :]
    jj = np.arange(128)[:, None]
    mL = (ii <= jj).astype(BF)                      # prev-bucket diag block
    mU = (ii >= jj).astype(BF)                      # own-bucket diag block
    # bl=0 halo-chunk full masks (band pattern; runtime-multiplied by hv)
    b0 = np.zeros((128, 1280), np.float32)
    for jc in range(4):
        w = WID[jc]
        blk = np.ones((128, w), np.float32)
        blk[:, 128 * jc:] = (ii <= jj).astype(np.float32)
        b0[:, B0OFF[jc]:B0OFF[jc] + w] = blk
    return {
        "cwqt": wqt, "cwkt": wkt, "cwvt": wvt, "cwgt": wgt, "cwot": wot,
        "cqs": qs, "cks": ks, "conesc": np.ones((128, 1), BF),
        "conesr": np.ones((1, 128), BF), "cbg": bgc,
        "cmL": np.ascontiguousarray(mL), "cmU": np.ascontiguousarray(mU),
        "cB0": np.ascontiguousarray(b0.astype(BF)),
    }


def build_nc(Wq=None, Wkv=None, q_scale=None, k_scale=None, Wg=None,
             bg=None, Wo=None, **_ignored):
    consts = _const_arrays(Wq, Wkv, q_scale, k_scale, Wg, bg, Wo)

    nc = bacc.Bacc("TRN2", target_bir_lowering=False, debug=False,
                   num_devices=NCORES)

    xz_d = nc.dram_tensor("xz", (D, TOK + NFLAG), BFD,
                          kind="ExternalInput").ap()
    y_d = nc.dram_tensor("y", (TOK, D), BFD, kind="ExternalOutput").ap()
    rsin_d = nc.dram_tensor("rsin", (NCORES * 128, DC * W), BFD,
                            kind="Internal").ap()
    rsout_d = nc.dram_tensor("rsout", (128, DC * W), BFD,
                             kind="Internal").ap()

    cap = {k: nc.inline_tensor(v, name=k).ap() for k, v in consts.items()}

    with tile.TileContext(nc) as tc:
        _emit(tc, nc, xz_d, rsin_d, rsout_d, cap, y_d)
    nc.compile()
    return nc, consts


def _emit(tc, nc, xz_d, rsin_d, rsout_d, cap, y_d):
    Exp = mybir.ActivationFunctionType.Exp
    Rsqrt = mybir.ActivationFunctionType.Rsqrt
    Sigmoid = mybir.ActivationFunctionType.Sigmoid
    Square = mybir.ActivationFunctionType.Square
    MUL = mybir.AluOpType.mult

    from contextlib import ExitStack
    ctx = ExitStack()
    with ctx:
        persist = ctx.enter_context(tc.tile_pool(name="persist", bufs=1))
        wpool = ctx.enter_context(tc.tile_pool(name="wpool", bufs=2))
        scr = ctx.enter_context(tc.tile_pool(name="scr", bufs=3))

        # ---- persistent tiles -------------------------------------------
        kT = persist.tile([128, H, EXT], BFD)        # [dh, h, ext_t]
        vS = persist.tile([128, EXT // 128, H * DH], BFD)  # [t%128, tblk, c]
        qT = persist.tile([128, H, TOK], BFD)        # [dh, h, own_t]
        gT = persist.tile([H, TOK], BFD)             # gates [h, own_t]
        gRow = persist.tile([1, H * TOK], BFD)       # gates on partition 0
        qs_t = persist.tile([1, DH], BFD, tag="c_qs")
        ks_t = persist.tile([1, DH], BFD, tag="c_ks")
        ones_c = persist.tile([128, 1], BFD, tag="c_oc")
        ones_r = persist.tile([1, 128], BFD, tag="c_or")
        bg_t = persist.tile([H, 1], F32, tag="c_bg")
        wg_t = persist.tile([128, DC, H], BFD, tag="c_wg")
        mL = persist.tile([128, 128], BFD, tag="c_mL")
        mU = persist.tile([128, 128], BFD, tag="c_mU")
        cB0 = persist.tile([128, 1280], BFD, tag="c_B0")
        mask0 = persist.tile([128, 1280], BFD, tag="mask0")
        nc.sync.dma_start(qs_t[:], cap["cqs"][:])
        nc.sync.dma_start(ks_t[:], cap["cks"][:])
        nc.sync.dma_start(ones_c[:], cap["conesc"][:])
        nc.sync.dma_start(ones_r[:], cap["conesr"][:])
        nc.sync.dma_start(bg_t[:], cap["cbg"][:])
        nc.sync.dma_start(wg_t[:], _r128(cap["cwgt"]))
        nc.sync.dma_start(mL[:], cap["cmL"][:])
        nc.sync.dma_start(mU[:], cap["cmU"][:])
        nc.sync.dma_start(cB0[:], cap["cB0"][:])

        # ---- weight tiles: ring of 4 slots, each 4 d-chunks (8KB/part) --
        def load_w(capname):
            tiles = []
            for i in range(4):
                t = wpool.tile([128, 4, H * DH], BFD, tag="w", bufs=6)
                nc.sync.dma_start(t[:],
                                  _r128(cap[capname])[:, 4 * i:4 * i + 4, :])
                tiles.append(t)
            return tiles

        def wsl(tiles, dc, lo, size):
            return tiles[dc // 4][:, dc % 4, lo:lo + size]

        wk = load_w("cwkt")
        wv = load_w("cwvt")

        eps_t = persist.tile([1, 1], F32, tag="c_eps")
        nc.gpsimd.memset(eps_t[:], 1e-12)

        def norm_drain(ppsum, psum_tile, scale_row, out_slice):
            """l2norm columns of psum (dh, 512), scale, write bf16."""
            sq = scr.tile([128, 512], BFD, tag="sq")
            nc.scalar.activation(sq[:], psum_tile[:], Square)
            ssp = ppsum.tile([1, 512], F32, tag="pnarrow", bufs=2)
            nc.tensor.matmul(ssp[:], ones_c[:], sq[:], start=True, stop=True)
            srt = scr.tile([1, 512], F32, tag="srt", bufs=2)
            nc.scalar.activation(srt[:], ssp[:],
                                 mybir.ActivationFunctionType.Sqrt,
                                 bias=eps_t[:])
            nc.vector.reciprocal(srt[:], srt[:])
            rn = scr.tile([1, 512], BFD, tag="rn", bufs=2)
            nc.vector.tensor_copy(rn[:], srt[:])
            obp = ppsum.tile([128, 512], F32, tag="pouter", bufs=2)
            nc.tensor.matmul(obp[:], scale_row[:], rn[:], start=True,
                             stop=True)
            osb = scr.tile([128, 512], BFD, tag="osb")
            nc.vector.tensor_copy(osb[:], obp[:])
            nc.vector.tensor_tensor(out_slice, psum_tile[:], osb[:], MUL)

        with (tc.tile_pool(name="xhpool", bufs=1) as xhpool,
              tc.tile_pool(name="xpool", bufs=1) as xpool,
              tc.tile_pool(name="ppsum", bufs=1, space="PSUM") as ppsum):
            xhalo = xhpool.tile([128, DC * W], BFD)  # exchanged halo, d-major
            # ---- load own x (d-major) and the flag columns --------------
            xt = []
            for dc in range(DC):
                t = xpool.tile([128, TOK], BFD, tag="xt", bufs=DC,
                               name=f"xt{dc}")
                nc.sync.dma_start(t[:], _r128(xz_d)[:, dc, 0:TOK])
                xt.append(t)
            fl = xpool.tile([128, NFLAG], BFD, tag="fl")
            nc.sync.dma_start(fl[:], _r128(xz_d)[:, 0, TOK:TOK + NFLAG])
            flf = xpool.tile([128, NFLAG], F32, tag="flf")
            nc.vector.tensor_copy(flf[:], fl[:])

            def xsl(dc, lo, size):
                """EXT-coordinate slice of x: halo (0:512) or own."""
                if lo < W:
                    assert lo + size <= W
                    return xhalo[:, dc * W + lo:dc * W + lo + size]
                return xt[dc][:, lo - W:lo - W + size]

            # ---- RS halo exchange (emitted first; runs on TOPSP/SDMA) ---
            for s in range(NCORES):
                for dc in range(DC):
                    st = xpool.tile([128, 512], BFD, tag="rsst", bufs=2)
                    nc.vector.tensor_scalar_mul(
                        st[:], xt[dc][:, W:TOK], flf[:, s:s + 1])
                    nc.sync.dma_start(
                        rsin_d[128 * s:128 * (s + 1),
                               W * dc:W * (dc + 1)], st[:])
            nc.gpsimd.collective_compute(
                "ReduceScatter", mybir.AluOpType.add,
                ins=[rsin_d[:]], outs=[rsout_d[:]],
                replica_groups=[list(range(NCORES))])
            nc.sync.dma_start(xhalo[:], rsout_d[:])

            # ---- own-region k projection + l2norm ------------------------
            for h in range(H):
                pks = [ppsum.tile([128, 512], F32, tag="pk", bufs=4,
                                  name=f"pk{h}_{t3}")
                       for t3 in range(1, 3)]
                for dc in range(DC):
                    for i, t3 in enumerate((1, 2)):
                        nc.tensor.matmul(
                            pks[i][:], wsl(wk, dc, DH * h, DH),
                            xsl(dc, 512 * t3, 512),
                            start=(dc == 0), stop=(dc == DC - 1))
                for i, t3 in enumerate((1, 2)):
                    norm_drain(ppsum, pks[i], ks_t,
                               kT[:, h, 512 * t3:512 * (t3 + 1)])

            # ---- own-region v projection (token-major) -------------------
            for tb in range(4, EXT // 128):
                pvs = [ppsum.tile([128, 512], F32, tag="pk", bufs=4,
                                  name=f"pv{tb}_{i}")
                       for i in range(2)]
                for dc in range(DC):
                    for cb in range(2):
                        nc.tensor.matmul(
                            pvs[cb][:], xsl(dc, 128 * tb, 128),
                            wsl(wv, dc, 512 * cb, 512),
                            start=(dc == 0), stop=(dc == DC - 1))
                for cb in range(2):
                    nc.vector.tensor_copy(
                        vS[:, tb, 512 * cb:512 * (cb + 1)], pvs[cb][:])

            # ---- gates ---------------------------------------------------
            for t2 in range(TOK // 512):
                pg = ppsum.tile([H, 512], F32, tag="pnarrow", bufs=2)
                for dc in range(DC):
                    nc.tensor.matmul(
                        pg[:], wg_t[:, dc, :],
                        xt[dc][:, 512 * t2:512 * (t2 + 1)],
                        start=(dc == 0), stop=(dc == DC - 1))
                nc.scalar.activation(gT[:, 512 * t2:512 * (t2 + 1)], pg[:],
                                     Sigmoid, bias=bg_t[:])
            for h in range(H):
                nc.sync.dma_start(gRow[0:1, h * TOK:(h + 1) * TOK],
                                  gT[h:h + 1, :])

            # ---- q projection + l2norm (recycles wk's slot) --------------
            wq = load_w("cwqt")
            for h in range(H):
                pqs = [ppsum.tile([128, 512], F32, tag="pk", bufs=4,
                                  name=f"pq{h}_{t2}")
                       for t2 in range(TOK // 512)]
                for dc in range(DC):
                    for t2 in range(TOK // 512):
                        nc.tensor.matmul(
                            pqs[t2][:], wsl(wq, dc, DH * h, DH),
                            xt[dc][:, 512 * t2:512 * (t2 + 1)],
                            start=(dc == 0), stop=(dc == DC - 1))
                for t2 in range(TOK // 512):
                    norm_drain(ppsum, pqs[t2], qs_t,
                               qT[:, h, 512 * t2:512 * (t2 + 1)])

            # ---- halo-region k/v projections (wait on the RS) ------------
            # bl=0 masks: band pattern times the halo-valid flag
            nc.vector.tensor_scalar_mul(mask0[:], cB0[:], flf[:, 8:9])
            # fresh weight copies: the own-phase wk/wv slots must release
            # early (wv/wq loads recycle them); reusing them here would put
            # their release after work that queues behind those loads.
            wk2 = load_w("cwkt")
            for h in range(H):
                pkh = ppsum.tile([128, 512], F32, tag="pk", bufs=4,
                                 name=f"pkh{h}")
                for dc in range(DC):
                    nc.tensor.matmul(
                        pkh[:], wsl(wk2, dc, DH * h, DH),
                        xsl(dc, 0, 512),
                        start=(dc == 0), stop=(dc == DC - 1))
                norm_drain(ppsum, pkh, ks_t, kT[:, h, 0:512])
            wv2 = load_w("cwvt")
            for tb in range(4):
                pvh = [ppsum.tile([128, 512], F32, tag="pk", bufs=4,
                                  name=f"pvh{tb}_{i}")
                       for i in range(2)]
                for dc in range(DC):
                    for cb in range(2):
                        nc.tensor.matmul(
                            pvh[cb][:], xsl(dc, 128 * tb, 128),
                            wsl(wv2, dc, 512 * cb, 512),
                            start=(dc == 0), stop=(dc == DC - 1))
                for cb in range(2):
                    nc.vector.tensor_copy(
                        vS[:, tb, 512 * cb:512 * (cb + 1)], pvh[cb][:])

        # xpool closed: its SBUF is reused by the attention pool below.
        wot = []
        for i in range(4):
            t = wpool.tile([128, 2, D], BFD, tag="w", bufs=6)
            nc.sync.dma_start(t[:], _r128(cap["cwot"])[:, 2 * i:2 * i + 2, :])
            wot.append(t)

        with (tc.tile_pool(name="attn", bufs=1) as apool,
              tc.tile_pool(name="apsum", bufs=1, space="PSUM") as apsum):
            oT = apool.tile([128, H, TOK], BFD)       # [dh, h, own_t]

            for bl in (1, 0):
                for h in range(H):
                    pms = []
                    for jc in range(8):
                        w, io, dg = WID[jc], IOFF[jc], DIAG[jc]
                        sim = apsum.tile([128, 512], F32, tag="sim", bufs=2)
                        nc.tensor.matmul(
                            sim[:, :w],
                            kT[:, h, 512 * bl + 128 * jc:
                                     512 * bl + 128 * (jc + 1)],
                            qT[:, h, 512 * bl + io:512 * bl + io + w],
                            start=True, stop=True)
                        pm = apool.tile([128, 512], BFD, tag="pm", bufs=8)
                        nc.scalar.activation(pm[:, :w], sim[:, :w], Exp)
                        if bl == 0 and jc < 4:
                            nc.vector.tensor_tensor(
                                pm[:, :w], pm[:, :w],
                                mask0[:, B0OFF[jc]:B0OFF[jc] + w], MUL)
                        else:
                            mt = mL if jc < 4 else mU
                            nc.vector.tensor_tensor(
                                pm[:, dg:dg + 128], pm[:, dg:dg + 128],
                                mt[:], MUL)
                        pms.append(pm)
                    ops = apsum.tile([128, 512], F32, tag="po", bufs=2)
                    ssp = apsum.tile([1, 512], F32, tag="pss", bufs=2)
                    for jc in range(8):
                        w, io = WID[jc], IOFF[jc]
                        nc.tensor.matmul(
                            ops[:, io:io + w],
                            vS[:, 4 * bl + jc, DH * h:DH * (h + 1)],
                            pms[jc][:, :w], start=(jc == 0), stop=(jc == 7))
                        nc.tensor.matmul(
                            ssp[:, io:io + w], ones_c[:], pms[jc][:, :w],
                            start=(jc == 0), stop=(jc == 7))
                    rr = apool.tile([1, 512], F32, tag="rr", bufs=2)
                    nc.vector.reciprocal(rr[:], ssp[:])
                    rg = apool.tile([1, 512], BFD, tag="rg", bufs=2)
                    nc.vector.tensor_tensor(
                        rg[:], rr[:],
                        gRow[0:1, h * TOK + 512 * bl:h * TOK + 512 * bl + 512],
                        MUL)
                    rgp = apsum.tile([128, 512], F32, tag="prgb", bufs=1)
                    nc.tensor.matmul(rgp[:], ones_r[:], rg[:], start=True,
                                     stop=True)
                    rgb = apool.tile([128, 512], BFD, tag="rgb", bufs=2)
                    nc.vector.tensor_copy(rgb[:], rgp[:])
                    nc.vector.tensor_tensor(
                        oT[:, h, 512 * bl:512 * (bl + 1)], ops[:], rgb[:],
                        MUL)

                # ---- output projection for this bucket -------------------
                for tq in range(4):
                    tck = 4 * bl + tq
                    ysb = apool.tile([128, D], BFD, tag="ysb", bufs=2)
                    for do in range(4):
                        yp = apsum.tile([128, 512], F32, tag="py", bufs=1)
                        for h in range(H):
                            nc.tensor.matmul(
                                yp[:],
                                oT[:, h, 128 * tck:128 * (tck + 1)],
                                wot[h // 2][:, h % 2,
                                            512 * do:512 * (do + 1)],
                                start=(h == 0), stop=(h == H - 1))
                        nc.vector.tensor_copy(ysb[:, 512 * do:512 * (do + 1)],
                                              yp[:])
                    nc.sync.dma_start(_r128(y_d)[:, tck, :], ysb[:])


def make_core_inputs(x, **_ignored):
    """Host-side sharding + layout prep. Returns list of 8 input dicts."""
    x = np.asarray(x, np.float32)
    in_maps = []
    per_core = B * N // NCORES
    for c in range(NCORES):
        g0 = c * per_core
        b_idx, t0 = g0 // N, g0 % N
        xz = np.empty((D, TOK + NFLAG), BF)
        xz[:, :TOK] = np.ascontiguousarray(
            x[b_idx, t0:t0 + TOK].T).astype(BF)
        flags = np.zeros(NFLAG, np.float32)
        flags[(c + 1) % NCORES] = 1.0        # RS slot for my tail
        flags[8] = 0.0 if t0 == 0 else 1.0   # halo valid
        xz[:, TOK:] = flags[None, :].astype(BF)
        in_maps.append({"xz": xz})
    return in_maps


def make_runner(nc, in_maps):
    """Persistent jitted executor.

    Binds ONLY the real ExternalInputs as operands (outputs are allocated by
    PJRT, not shipped as pre-zeroed donated buffers — the kernel writes every
    output element, so zero-init is unnecessary and shipping the zero buffers
    per call costs ~1ms of wire time under the axon tunnel).
    """
    import jax
    from jax.sharding import Mesh, PartitionSpec
    try:
        from jax.experimental.shard_map import shard_map
    except ImportError:
        from jax.shard_map import shard_map
    from concourse.bass2jax import (_bass_exec_p, install_neuronx_cc_hook,
                                    partition_id_tensor)

    install_neuronx_cc_hook()
    partition_name = (nc.partition_id_tensor.name
                      if nc.partition_id_tensor else None)
    in_names, out_names, out_avals = [], [], []
    for alloc in nc.m.functions[0].allocations:
        if not isinstance(alloc, mybir.MemoryLocationSet):
            continue
        name = alloc.memorylocations[0].name
        if alloc.kind == "ExternalInput":
            if name != partition_name:
                in_names.append(name)
        elif alloc.kind == "ExternalOutput":
            out_names.append(name)
            out_avals.append(jax.core.ShapedArray(
                tuple(alloc.tensor_shape), mybir.dt.np(alloc.dtype)))
    n_params = len(in_names)
    all_names = list(in_names)
    if partition_name is not None:
        all_names.append(partition_name)

    def _body(*args):
        operands = list(args)
        if partition_name is not None:
            operands.append(partition_id_tensor())
        outs = _bass_exec_p.bind(
            *operands, out_avals=tuple(out_avals), in_names=tuple(all_names),
            out_names=tuple(out_names), lowering_input_output_aliases=(),
            sim_require_finite=False, sim_require_nnan=False, nc=nc)
        return tuple(outs)

    devices = jax.devices()[:NCORES]
    mesh = Mesh(np.asarray(devices), ("core",))
    run = jax.jit(
        shard_map(_body, mesh=mesh,
                  in_specs=(PartitionSpec("core"),) * n_params,
                  out_specs=(PartitionSpec("core"),) * len(out_names),
                  check_rep=False),
        keep_unused=True)
    concat_in = [np.concatenate([np.asarray(in_maps[c][nm])
                                 for c in range(NCORES)], axis=0)
                 for nm in in_names]
    args = [jax.device_put(a) for a in concat_in]
    return run, args


def assemble_output(out_np):
    """out_np: list with the concatenated 'y' array -> full (B, N, D) f32."""
    y = out_np[0]
    out = np.empty((B, N, D), np.float32)
    per_core = B * N // NCORES
    for c in range(NCORES):
        g0 = c * per_core
        out[g0 // N, g0 % N:g0 % N + TOK] = \
            y[c * TOK:(c + 1) * TOK].astype(np.float32)
    return out


_NC_CACHE = None        # (weight_hash, nc, run, weight_ids)


def _whash(inputs):
    h = hashlib.sha256()
    for k in ("Wq", "Wkv", "q_scale", "k_scale", "Wg", "bg", "Wo"):
        h.update(np.ascontiguousarray(np.asarray(inputs[k], np.float32)))
    return h.hexdigest()


def kernel(**inputs):
    global _NC_CACHE
    import jax
    wids = tuple(id(inputs[k]) for k in
                 ("Wq", "Wkv", "q_scale", "k_scale", "Wg", "bg", "Wo"))
    if _NC_CACHE is None or _NC_CACHE[3] != wids:
        wh = _whash(inputs)
        if _NC_CACHE is None or _NC_CACHE[0] != wh:
            nc, _ = build_nc(**inputs)
            in_maps = make_core_inputs(**inputs)
            run, args = make_runner(nc, in_maps)
            _NC_CACHE = (wh, nc, run, wids)
        else:
            _NC_CACHE = (_NC_CACHE[0], _NC_CACHE[1], _NC_CACHE[2], wids)
    _, nc, run, _ = _NC_CACHE
    in_maps = make_core_inputs(**inputs)
    concat = np.concatenate([in_maps[c]["xz"] for c in range(NCORES)], axis=0)
    out = run(jax.device_put(concat))
    out_np = [np.asarray(o) for o in out]
    return assemble_output(out_np)


if __name__ == "__main__":
    rng = np.random.default_rng(0)
    nc, _ = build_nc(
        Wq=rng.standard_normal((H * DH, D), np.float32) * 0.02,
        Wkv=rng.standard_normal((2 * H * DH, D), np.float32) * 0.02,
        q_scale=np.ones(DH, np.float32), k_scale=np.ones(DH, np.float32),
        Wg=rng.standard_normal((H, D), np.float32) * 0.02,
        bg=np.zeros(H, np.float32),
        Wo=rng.standard_normal((D, H * DH), np.float32) * 0.02)
    print("built ok")
